# revision 1
# baseline (speedup 1.0000x reference)
"""BasicGCN (2-layer GCN, 100K nodes / 3.2M edges) on 8 Trainium2 NeuronCores.

Strategy (node/dst sharding, graph-parallel):
  - Pad nodes to NPAD = 100352 = 8 * 12544; core c owns dst rows
    [c*12544, (c+1)*12544).
  - Host preprocessing (index-space only): degrees/dinv, per-core edge
    streams bucketed by (dst-tile-quad, src-group, dst-tile), self-loops
    appended as real edges, per-(tile,group) slot quotas equalized across
    cores so one SPMD program serves all 8 cores.
  - Device per core:
      phase 1 (dense, replicated): h1p = dinv * (x @ W1) for ALL nodes
        (fp32r matmuls), written to an HBM table [NPAD, 256] bf16.
      phase 2 (layer-1 aggregation): dma_gather h1p rows (512B each) in
        <=1024-row calls spanning a quad of dst tiles, build one-hot S
        blocks on DVE (S[e,d] = dst_local[e]==d), segment-sum via PE bf16
        matmuls into per-tile f32 PSUM accumulators; epilogue computes
        h2p_own = dinv_d * (relu(dinv_d*agg + b1) @ W2)  -> bf16 shard
        table [12544, 128] (64 data + 64 zero pad).
      AllGather shards -> h2p_full [NPAD, 128] bf16 (Shared DRAM).
      phase 3 (layer-2 aggregation): same gather/S/matmul with 256B rows,
        epilogue log_softmax (f32) -> out shard [12544, 64].
  - Host: concatenate 8 shards, trim to [100000, 64].

Gather tables are split into 4 row-groups of NPAD/4 = 25088 rows so the
int16 gather indices stay in range; each dma_gather call is capped at
QMAX=1024 indices (the q7 firmware breaks above ~1024) and spans the
quad's whole per-group run to keep calls full (SWDGE fixed cost is the
main Pool-engine expense).
"""

import numpy as np

import concourse.bacc as bacc
import concourse.bass as bass
import concourse.mybir as mybir
import concourse.tile as tile
from concourse.bass_utils import run_bass_kernel_spmd

F32 = mybir.dt.float32
F32R = mybir.dt.float32r
BF16 = mybir.dt.bfloat16
I16 = mybir.dt.int16
NP_BF16 = mybir.dt.np(BF16)
AF = mybir.ActivationFunctionType
ALU = mybir.AluOpType

N_CORES = 8
PAD_DSTLOC = 1000.0  # sentinel dst-local for padding slots -> zero S column
QMAX = 1024  # max num_idxs per dma_gather call (HW limit is in (1024, 1280])
QT = 4       # dst tiles per gather bucket (quad)


def make_cfg(n_nodes=100000, d_in=256, d_hid=256, d_out=64, shard_tiles=98,
             n_groups=4, sup=512):
    shard = shard_tiles * 128
    npad = N_CORES * shard
    assert npad % n_groups == 0
    gr = npad // n_groups
    assert gr <= 32768
    assert npad % sup == 0
    assert n_nodes <= npad
    return dict(N=n_nodes, NPAD=npad, SHARD=shard, NT=shard_tiles,
                NG=n_groups, GR=gr, D_IN=d_in, D_HID=d_hid, D_OUT=d_out,
                SUP=sup)


FULL_CFG = make_cfg()


def _build_schedule(quota, nt, ng):
    """Gather-call schedule over (quad, group) runs.

    Returns (calls, blk_tile, call_off_flat, slot_total):
      calls: list of (g, icol, q) in stream order, q <= QMAX, all %128==0
      blk_tile: tile id per 128-slot block, in stream order
      call_off_flat[t*ng+g]: slot offset of the (t,g) section
    """
    call_off_flat = np.zeros(nt * ng, np.int64)
    blk_tile = []
    calls = []
    off = 0
    for qd in range(0, nt, QT):
        tiles = range(qd, min(qd + QT, nt))
        for g in range(ng):
            total = 0
            for t in tiles:
                q = int(quota[t, g])
                call_off_flat[t * ng + g] = off + total
                blk_tile.extend([t] * (q // 128))
                total += q
            if total == 0:
                continue
            nblk = total // 128
            nch = (total + QMAX - 1) // QMAX
            base, rem = divmod(nblk, nch)
            o = off
            for i in range(nch):
                q = (base + (1 if i < rem else 0)) * 128
                calls.append((g, o, q))
                o += q
            off += total
    return calls, blk_tile, call_off_flat, off


# --------------------------------------------------------------------------
# Host preprocessing
# --------------------------------------------------------------------------

def preprocess(x, edge_index, W1, b1, W2, b2, cfg):
    N, NPAD, SHARD, NT, NG, GR = (cfg["N"], cfg["NPAD"], cfg["SHARD"],
                                  cfg["NT"], cfg["NG"], cfg["GR"])
    D_IN, D_HID, D_OUT = cfg["D_IN"], cfg["D_HID"], cfg["D_OUT"]

    x = np.asarray(x, np.float32)
    edge_index = np.asarray(edge_index)
    src = edge_index[0].astype(np.int64)
    dst = edge_index[1].astype(np.int64)

    deg = np.bincount(dst, minlength=N).astype(np.float32) + 1.0
    dinv = 1.0 / np.sqrt(deg)
    dinv_pad = np.zeros(NPAD, np.float32)
    dinv_pad[:N] = dinv

    # self loops appended as regular edges
    loops = np.arange(N, dtype=np.int64)
    src_all = np.concatenate([src, loops])
    dst_all = np.concatenate([dst, loops])
    E = src_all.shape[0]

    c_of = dst_all // SHARD
    t_of = (dst_all % SHARD) // 128
    d_of = (dst_all % 128).astype(np.float32)
    g_of = src_all // GR
    srcg = (src_all % GR).astype(np.int16)

    key = (c_of * NT + t_of) * NG + g_of
    order = np.argsort(key, kind="stable")
    counts = np.bincount(key, minlength=N_CORES * NT * NG)
    quota = counts.reshape(N_CORES, NT, NG).max(axis=0)
    quota = ((quota + 127) // 128) * 128  # round up to whole 128-slot blocks

    calls, blk_tile, call_off_flat, slot_total = _build_schedule(
        quota, NT, NG)

    # slot position of each edge inside its core's stream
    csum = np.zeros(N_CORES * NT * NG + 1, np.int64)
    np.cumsum(counts, out=csum[1:])
    sorted_key = key[order]
    rank = np.arange(E, dtype=np.int64) - csum[sorted_key]
    tg = t_of[order] * NG + g_of[order]
    slot = call_off_flat[tg] + rank
    core = c_of[order]

    idx_arr = np.zeros((N_CORES, slot_total), np.int16)  # pad -> row 0
    dl_arr = np.full((N_CORES, slot_total), PAD_DSTLOC, np.float32)
    idx_arr[core, slot] = srcg[order]
    dl_arr[core, slot] = d_of[order]

    # per-section wrapping: idx wrapped [16, Q/16] (replicated to 128
    # parts), dstloc wrapped [128, Q/128]
    idxcols = slot_total // 16
    nb = slot_total // 128
    idx_sb = np.zeros((N_CORES, 16, idxcols), np.int16)
    dl_sb = np.full((N_CORES, 128, nb), PAD_DSTLOC, np.float32)
    for tgi in range(NT * NG):
        q = int(quota.reshape(-1)[tgi])
        if q == 0:
            continue
        o = int(call_off_flat[tgi])
        seg = idx_arr[:, o:o + q].reshape(N_CORES, q // 16, 16)
        idx_sb[:, :, o // 16:(o + q) // 16] = seg.transpose(0, 2, 1)
        dseg = dl_arr[:, o:o + q].reshape(N_CORES, q // 128, 128)
        dl_sb[:, :, o // 128:(o + q) // 128] = dseg.transpose(0, 2, 1)
    idx_sb = np.tile(idx_sb, (1, 8, 1))  # replicate to 128 partitions

    # dense inputs
    xp = np.zeros((NPAD, D_IN), np.float32)
    xp[:N] = x
    xT = np.ascontiguousarray(xp.T)  # [D_IN, NPAD]

    ntile = NPAD // 128
    dinv_nodes = np.ascontiguousarray(
        dinv_pad.reshape(ntile, 128).T)  # [128, ntile]
    dinv_dst = np.stack([dinv_nodes[:, c * NT:(c + 1) * NT]
                         for c in range(N_CORES)])  # [8, 128, NT]

    iota = np.tile(np.arange(128), (128, 1)).astype(NP_BF16)
    ident = np.eye(128, dtype=np.float32)
    b1bc = np.ascontiguousarray(
        np.broadcast_to(np.asarray(b1, np.float32), (128, D_HID)))
    b2bc = np.ascontiguousarray(
        np.broadcast_to(np.asarray(b2, np.float32), (128, D_OUT)))

    common = dict(xT=xT.astype(NP_BF16), W1=np.asarray(W1, NP_BF16),
                  W2=np.asarray(W2, np.float32), b1bc=b1bc, b2bc=b2bc,
                  iota=iota, ident=ident, dinv_nodes=dinv_nodes)
    in_maps = []
    for c in range(N_CORES):
        m = dict(common)
        m["dinv_dst"] = np.ascontiguousarray(dinv_dst[c])
        m["idx_sb"] = np.ascontiguousarray(idx_sb[c])
        m["dstloc"] = np.ascontiguousarray(dl_sb[c])
        in_maps.append(m)

    meta = dict(quota=quota, idxcols=idxcols, nb=nb, calls=calls,
                blk_tile=blk_tile)
    return in_maps, meta


# --------------------------------------------------------------------------
# Device program
# --------------------------------------------------------------------------

def build_program(cfg, meta, with_collective=True, phases=(1, 2, 3)):
    NPAD, NT, NG, GR, SUP = (cfg["NPAD"], cfg["NT"], cfg["NG"], cfg["GR"],
                             cfg["SUP"])
    D_IN, D_HID, D_OUT = cfg["D_IN"], cfg["D_HID"], cfg["D_OUT"]
    SHARD = cfg["SHARD"]
    quota = meta["quota"]
    idxcols, nb = meta["idxcols"], meta["nb"]
    calls, blk_tile = meta["calls"], meta["blk_tile"]
    ntile = NPAD // 128
    KI = D_IN // 128   # k-chunks for layer-1 matmul
    KH = D_HID // 128  # k-chunks for W2 matmul
    CMAX = QMAX // 128
    D_L2 = 2 * D_OUT  # layer-2 table row: 64 bf16 data + 64 bf16 zeros

    # first/last block of each tile (accumulation start/stop flags)
    first_blk = {}
    last_blk = {}
    for i, t in enumerate(blk_tile):
        first_blk.setdefault(t, i)
        last_blk[t] = i

    nc = bacc.Bacc("TRN2", target_bir_lowering=False, debug=False,
                   num_devices=N_CORES)

    xT_d = nc.dram_tensor("xT", [D_IN, NPAD], BF16, kind="ExternalInput")
    W1_d = nc.dram_tensor("W1", [D_IN, D_HID], BF16, kind="ExternalInput")
    W2_d = nc.dram_tensor("W2", [D_HID, D_OUT], F32, kind="ExternalInput")
    b1_d = nc.dram_tensor("b1bc", [128, D_HID], F32, kind="ExternalInput")
    b2_d = nc.dram_tensor("b2bc", [128, D_OUT], F32, kind="ExternalInput")
    iota_d = nc.dram_tensor("iota", [128, 128], BF16, kind="ExternalInput")
    ident_d = nc.dram_tensor("ident", [128, 128], F32, kind="ExternalInput")
    dinvn_d = nc.dram_tensor("dinv_nodes", [128, ntile], F32,
                             kind="ExternalInput")
    dinvd_d = nc.dram_tensor("dinv_dst", [128, NT], F32, kind="ExternalInput")
    idx_d = nc.dram_tensor("idx_sb", [128, idxcols], I16, kind="ExternalInput")
    dl_d = nc.dram_tensor("dstloc", [128, nb], F32, kind="ExternalInput")
    out_d = nc.dram_tensor("out", [SHARD, D_OUT], F32, kind="ExternalOutput")

    with tile.TileContext(nc) as tc:
        with (
            tc.tile_pool(name="const", bufs=1) as const,
            tc.tile_pool(name="dram", bufs=1, space="DRAM") as dram,
        ):
            h1p_g = [dram.tile([GR, D_HID], BF16, name=f"h1p{g}")
                     for g in range(NG)]
            h2own = dram.tile([SHARD, D_L2], BF16)
            h2full = dram.tile([NPAD, D_L2], BF16, addr_space="Shared")

            w1_sb = const.tile([128, KI, D_HID], BF16)
            for k in range(KI):
                nc.sync.dma_start(out=w1_sb[:, k, :],
                                  in_=W1_d.ap()[k * 128:(k + 1) * 128, :])
            w2_sb = const.tile([128, KH, D_OUT], F32)
            for k in range(KH):
                nc.sync.dma_start(out=w2_sb[:, k, :],
                                  in_=W2_d.ap()[k * 128:(k + 1) * 128, :])
            iota_sb = const.tile([128, 128], BF16)
            nc.sync.dma_start(out=iota_sb[:], in_=iota_d.ap())
            ident_sb = const.tile([128, 128], F32)
            nc.sync.dma_start(out=ident_sb[:], in_=ident_d.ap())
            b1_sb = const.tile([128, D_HID], F32)
            nc.sync.dma_start(out=b1_sb[:], in_=b1_d.ap())
            b2_sb = const.tile([128, D_OUT], F32)
            nc.sync.dma_start(out=b2_sb[:], in_=b2_d.ap())
            dinvn_sb = const.tile([128, ntile], F32)
            nc.sync.dma_start(out=dinvn_sb[:], in_=dinvn_d.ap())
            dinvd_sb = const.tile([128, NT], F32)
            nc.sync.dma_start(out=dinvd_sb[:], in_=dinvd_d.ap())
            idx_sb = const.tile([128, idxcols], I16)
            nc.sync.dma_start(out=idx_sb[:], in_=idx_d.ap())
            dl_sb = const.tile([128, nb], F32)
            nc.sync.dma_start(out=dl_sb[:], in_=dl_d.ap())

            # ---------------- phase 1: h1p = dinv * (x @ W1) --------------
            nsup = NPAD // SUP if 1 in phases else 0
            nsub = SUP // 128
            assert GR % SUP == 0
            sup_per_g = GR // SUP
            xT_r = xT_d.ap().rearrange("(k p) n -> p k n", p=128)
            h1p_g_r = [t.rearrange("(s u p) f -> s p u f", p=128, u=nsub)
                       for t in h1p_g]
            with (
                tc.tile_pool(name="p1x", bufs=4) as p1x,
                tc.tile_pool(name="p1o", bufs=4) as p1o,
                tc.tile_pool(name="p1ps", bufs=4, space="PSUM") as p1ps,
            ):
                for s in range(nsup):
                    xt = p1x.tile([128, KI, SUP], BF16, tag="xt")
                    nc.sync.dma_start(
                        out=xt[:], in_=xT_r[:, :, s * SUP:(s + 1) * SUP])
                    ot = p1o.tile([128, nsub, D_HID], BF16, tag="h1o")
                    for u in range(nsub):
                        ps = p1ps.tile([128, D_HID], F32, tag="p1")
                        for k in range(KI):
                            nc.tensor.matmul(
                                ps[:],
                                xt[:, k, u * 128:(u + 1) * 128],
                                w1_sb[:, k, :],
                                start=(k == 0), stop=(k == KI - 1))
                        gt = s * nsub + u
                        nc.scalar.activation(
                            ot[:, u, :], ps[:], AF.Copy,
                            scale=dinvn_sb[:, gt:gt + 1])
                    nc.sync.dma_start(
                        out=h1p_g_r[s // sup_per_g][s % sup_per_g],
                        in_=ot[:])

            # ---------------- phase 2: layer-1 agg + h2p shard ------------
            h2own_r = h2own.rearrange("(t p) f -> t p f", p=128)

            def agg_phase(table, elem, rhs_w, epilogue, mtag, stag, ptag):
                """Gather + one-hot-S + matmul accumulation over the
                precomputed quad-spanning call schedule."""
                blk = 0
                psums = {}
                with (
                    tc.tile_pool(name=mtag, bufs=10) as mpool,
                    tc.tile_pool(name=stag, bufs=8) as spool,
                    tc.tile_pool(name=ptag, bufs=6, space="PSUM") as apsum,
                    tc.tile_pool(name=ptag + "ep", bufs=3) as eppool,
                    tc.tile_pool(name=ptag + "ep2", bufs=1,
                                 space="PSUM") as eppsum,
                ):
                    for g, o, q in calls:
                        ncols = q // 128
                        mt = mpool.tile([128, CMAX, elem], BF16, tag="m")
                        nc.gpsimd.dma_gather(
                            mt[:, :ncols, :],
                            table(g),
                            idx_sb[:, o // 16:(o + q) // 16],
                            q, q, elem)
                        for j in range(ncols):
                            t = blk_tile[blk]
                            if blk == first_blk[t]:
                                psums[t] = apsum.tile(
                                    [128, rhs_w], F32, tag="agg",
                                    name="aggps")
                            st = spool.tile([128, 128], BF16, tag="s",
                                            name="stile")
                            nc.vector.tensor_scalar(
                                st[:], iota_sb[:], dl_sb[:, blk:blk + 1],
                                None, ALU.is_equal)
                            nc.tensor.matmul(
                                psums[t][:], st[:], mt[:, j, :rhs_w],
                                start=(blk == first_blk[t]),
                                stop=(blk == last_blk[t]))
                            if blk == last_blk[t]:
                                epilogue(t, psums.pop(t), eppool, eppsum)
                            blk += 1

            def epi1(t, ps, eppool, eppsum):
                o1 = eppool.tile([128, D_HID], F32, tag="o1")
                nc.vector.tensor_scalar(o1[:], ps[:], dinvd_sb[:, t:t + 1],
                                        None, ALU.mult)
                nc.vector.tensor_tensor(o1[:], o1[:], b1_sb[:], ALU.add)
                nc.scalar.activation(o1[:], o1[:], AF.Relu)
                tsb = eppool.tile([128, KH, 128], F32, tag="tsb")
                for k in range(KH):
                    tp = eppsum.tile([128, 128], F32, tag="tr")
                    nc.tensor.transpose(tp[:], o1[:, k * 128:(k + 1) * 128],
                                        ident_sb[:])
                    nc.vector.tensor_copy(tsb[:, k, :], tp[:])
                h2ps = eppsum.tile([128, D_OUT], F32, tag="h2")
                for k in range(KH):
                    nc.tensor.matmul(h2ps[:], tsb[:, k, :], w2_sb[:, k, :],
                                     start=(k == 0), stop=(k == KH - 1))
                h2sb = eppool.tile([128, D_L2], BF16, tag="h2sb")
                nc.vector.tensor_scalar(h2sb[:, :D_OUT], h2ps[:],
                                        dinvd_sb[:, t:t + 1], None, ALU.mult)
                nc.vector.memset(h2sb[:, D_OUT:], 0.0)
                nc.sync.dma_start(out=h2own_r[t], in_=h2sb[:])

            if 2 in phases:
                agg_phase(lambda g: h1p_g[g][:, :], D_HID, D_HID, epi1,
                          "m1", "s1", "ag1")

            # ---------------- AllGather h2 shards -------------------------
            if with_collective and 2 in phases:
                nc.gpsimd.collective_compute(
                    "AllGather", ALU.bypass,
                    replica_groups=[list(range(N_CORES))],
                    ins=[h2own.opt()], outs=[h2full.opt()])

            # ---------------- phase 3: layer-2 agg + log_softmax ----------
            out_r = out_d.ap().rearrange("(t p) f -> t p f", p=128)

            def epi2(t, ps, eppool, eppsum):
                t0 = eppool.tile([128, D_OUT], F32, tag="t0")
                nc.vector.tensor_scalar(t0[:], ps[:], dinvd_sb[:, t:t + 1],
                                        None, ALU.mult)
                nc.vector.tensor_tensor(t0[:], t0[:], b2_sb[:], ALU.add)
                nm = eppool.tile([128, 1], F32, tag="nm")
                nc.vector.tensor_reduce(nm[:], t0[:], mybir.AxisListType.X,
                                        ALU.max, negate=True)
                et = eppool.tile([128, D_OUT], F32, tag="et")
                se = eppool.tile([128, 1], F32, tag="se")
                nc.scalar.activation(et[:], t0[:], AF.Exp, bias=nm[:],
                                     accum_out=se[:])
                ls = eppool.tile([128, 1], F32, tag="ls")
                nc.scalar.activation(ls[:], se[:], AF.Ln)
                ot = eppool.tile([128, D_OUT], F32, tag="ot")
                nc.vector.tensor_scalar(ot[:], t0[:], nm[:], ls[:],
                                        ALU.add, ALU.subtract)
                nc.sync.dma_start(out=out_r[t], in_=ot[:])

            if 3 in phases:
                agg_phase(lambda g: h2full[g * GR:(g + 1) * GR, :], D_L2,
                          D_OUT, epi2, "m2", "s2", "ag2")

    nc.compile()
    return nc


# --------------------------------------------------------------------------
# Entry point
# --------------------------------------------------------------------------

def kernel(x, edge_index, W1, b1, W2, b2):
    cfg = FULL_CFG
    in_maps, meta = preprocess(x, edge_index, W1, b1, W2, b2, cfg)
    nc = build_program(cfg, meta)
    res = run_bass_kernel_spmd(nc, in_maps, core_ids=list(range(N_CORES)))
    shards = [res.results[c]["out"] for c in range(N_CORES)]
    full = np.concatenate(shards, axis=0)
    return full[:cfg["N"]].astype(np.float32)



# revision 17
# speedup vs baseline: 1.3288x; 1.3288x over previous
"""BasicGCN (2-layer GCN, 100K nodes / 3.2M edges) on 8 Trainium2 NeuronCores.

Strategy (node/dst sharding, graph-parallel, aggregate-first):
  - Pad nodes to NPAD = 100352; 784 dst tiles of 128 nodes are assigned to
    8 cores x 98 positions by a host-side balancing pass (minimizes the
    max-over-cores per-tile edge counts that one SPMD schedule must cover).
  - Layer 1 is computed aggregation-first:  out1 = (A_norm X) W1, so the
    gather table is just xs = dinv * x (prescaled on host, bf16) and the
    dense x@W1 phase of the old design disappears entirely.  The per-tile
    epilogue computes h1 = relu(dinv_d * (agg @ W1) + b1) and the layer-2
    table row h2 = dinv_d * (h1 @ W2) in one go.
  - Edge slots are bucketed per (core, tile-position), sorted by src within
    a bucket (alternating asc/desc so consecutive sections chain), padded
    to 128-multiples, and streamed through dma_gather calls of <= QMAX
    slots.  Each call gathers from a table slice starting at a per-call
    base row, so the int16 gather indices cover the whole node range with
    no group decomposition.  Bases are common across cores (SPMD); the
    schedule enforces max-over-cores span <= 32767 per call.
  - Segment-sum per 128-slot block via one-hot S matrices on DVE
    (S[e,d] = dst_local[e]==d) and PE bf16 matmuls into per-tile PSUM
    accumulators; both layers share the same slot schedule / idx table.
  - AllGather of the per-core h2 shard tables -> h2full [NPAD, 128] bf16.
  - Layer-2 aggregation the same way (256B rows); the log_softmax ln and
    final subtraction are deferred to a tail pass so the Activation engine
    loads the exp/ln tables O(1) times instead of per-tile.
  - Host: reorder the 8 gathered shards back to node order, trim.
"""

import numpy as np

import concourse.bacc as bacc
import concourse.bass as bass
import concourse.mybir as mybir
import concourse.tile as tile
from concourse.bass_utils import run_bass_kernel_spmd

F32 = mybir.dt.float32
BF16 = mybir.dt.bfloat16
I16 = mybir.dt.int16
NP_BF16 = mybir.dt.np(BF16)
AF = mybir.ActivationFunctionType
ALU = mybir.AluOpType

N_CORES = 8
PAD_DSTLOC = 1000.0  # sentinel dst-local for padding slots -> zero S column
QMAX = 1024          # max num_idxs per dma_gather call (HW limit ~1024)
SPAN = 32767         # max idx value (int16, non-negative)


def make_cfg(n_nodes=100000, d_in=256, d_hid=256, d_out=64, shard_tiles=98):
    shard = shard_tiles * 128
    npad = N_CORES * shard
    assert n_nodes <= npad
    return dict(N=n_nodes, NPAD=npad, SHARD=shard, NT=shard_tiles,
                D_IN=d_in, D_HID=d_hid, D_OUT=d_out)


FULL_CFG = make_cfg()


# --------------------------------------------------------------------------
# Host preprocessing
# --------------------------------------------------------------------------

def _assign_tiles(tile_tot, nt):
    """Partition the 8*nt global tiles into nt groups of 8 (one per slot
    position; members go to distinct cores), minimizing the padded
    sum-of-max slot quota.  1-D objective -> sorting by total is near
    optimal."""
    order = np.argsort(tile_tot, kind="stable")
    groups = order.reshape(nt, N_CORES)
    return groups


def _build_schedule(cnt_min_src, cnt_max_src, quota):
    """Chunk the common slot stream into gather calls.

    quota[p]: padded (128-mult) slot count of section p.
    cnt_min_src/cnt_max_src[p][b]: per-block (128 slots) min/max src over
    ALL cores for section p block b.
    Returns list of calls (offset, q, base) and blk_tile (position per
    block)."""
    calls = []
    blk_tile = []
    off = 0
    # flatten blocks in stream order with their tile position
    blocks = []
    for p, q in enumerate(quota):
        nb = q // 128
        for b in range(nb):
            blocks.append((p, cnt_min_src[p][b], cnt_max_src[p][b]))
            blk_tile.append(p)
    i = 0
    while i < len(blocks):
        base = blocks[i][1]
        hi = blocks[i][2]
        j = i + 1
        while j < len(blocks) and j - i < QMAX // 128:
            nb_ = min(base, blocks[j][1])
            nh = max(hi, blocks[j][2])
            if nh - nb_ > SPAN:
                break
            base, hi = nb_, nh
            j += 1
        calls.append((off, (j - i) * 128, int(base)))
        off += (j - i) * 128
        i = j
    return calls, blk_tile, off


def preprocess(x, edge_index, W1, b1, W2, b2, cfg):
    N, NPAD, SHARD, NT = cfg["N"], cfg["NPAD"], cfg["SHARD"], cfg["NT"]
    D_IN, D_HID, D_OUT = cfg["D_IN"], cfg["D_HID"], cfg["D_OUT"]

    x = np.asarray(x, np.float32)
    edge_index = np.asarray(edge_index)
    src = edge_index[0].astype(np.int64)
    dst = edge_index[1].astype(np.int64)

    deg = np.bincount(dst, minlength=N).astype(np.float32) + 1.0
    dinv = 1.0 / np.sqrt(deg)
    dinv_pad = np.zeros(NPAD, np.float32)
    dinv_pad[:N] = dinv

    # self loops appended as regular edges
    loops = np.arange(N, dtype=np.int64)
    src_all = np.concatenate([src, loops])
    dst_all = np.concatenate([dst, loops])
    E = src_all.shape[0]

    gt_of = dst_all // 128            # global dst tile
    d_of = (dst_all % 128).astype(np.float32)

    # ---- balanced assignment of global tiles to (core, position) ----
    ntile_g = NPAD // 128
    tile_tot = np.bincount(gt_of, minlength=ntile_g)
    groups = _assign_tiles(tile_tot, NT)        # [NT, 8] global tile ids
    tile_core = np.empty(ntile_g, np.int64)
    tile_pos = np.empty(ntile_g, np.int64)
    for p in range(NT):
        for c in range(N_CORES):
            tile_core[groups[p, c]] = c
            tile_pos[groups[p, c]] = p

    c_of = tile_core[gt_of]
    p_of = tile_pos[gt_of]

    # node -> position in the gathered/AllGathered table (h2full row)
    tile_ids = np.arange(ntile_g)
    newbase = tile_core[tile_ids] * SHARD + tile_pos[tile_ids] * 128
    node_newpos = newbase[np.arange(NPAD) // 128] + (np.arange(NPAD) % 128)

    # L1 table (xs) is laid out in NEW order too so both layers share idx
    # values: gather row for a src node = node_newpos[src].
    s_of = node_newpos[src_all]

    # ---- per-(core, position) sections, sorted by src (boustrophedon) ----
    key = (c_of * NT + p_of)
    # sort edges by (core, position, +-src)
    sgn_src = np.where((p_of % 2) == 0, s_of, NPAD - 1 - s_of)
    order = np.lexsort((sgn_src, key))
    counts = np.bincount(key, minlength=N_CORES * NT).reshape(N_CORES, NT)
    quota = counts.max(axis=0)
    quota = ((quota + 127) // 128) * 128
    slot_total = int(quota.sum())

    sec_off = np.concatenate([[0], np.cumsum(quota)])[:-1]

    csum = np.zeros(N_CORES * NT + 1, np.int64)
    np.cumsum(counts.reshape(-1), out=csum[1:])
    rank = np.arange(E, dtype=np.int64) - csum[key[order]]
    # progress-aligned placement: core c's k-th edge of a section goes to
    # slot floor(k*q/cnt), so all cores sweep the src range in lockstep and
    # every 128-slot block has a tight cross-core src span.
    q_e = quota[p_of[order]]
    cnt_e = counts[c_of[order], p_of[order]]
    slot = sec_off[p_of[order]] + (rank * q_e) // cnt_e
    core = c_of[order]

    src_arr = np.full((N_CORES, slot_total), -1, np.int64)
    dl_arr = np.full((N_CORES, slot_total), PAD_DSTLOC, np.float32)
    src_arr[core, slot] = s_of[order]
    dl_arr[core, slot] = d_of[order]

    # pad slots: fill with a neighboring real src of the same section (same
    # core), or the quota-defining core's stream when the core has no edges
    # there.  Pads gather a live row; their S column is dead via the dstloc
    # sentinel.
    for p in range(NT):
        o = int(sec_off[p])
        q = int(quota[p])
        if q == 0:
            continue
        refc = int(np.argmax(counts[:, p]))
        for c in range(N_CORES):
            seg = src_arr[c, o:o + q]
            real = np.nonzero(seg >= 0)[0]
            if len(real) == 0:
                continue  # second pass below
            pos = np.clip(np.searchsorted(real, np.arange(q), side="right")
                          - 1, 0, len(real) - 1)
            src_arr[c, o:o + q] = seg[real[pos]]
        ref = src_arr[refc, o:o + q]
        for c in range(N_CORES):
            if counts[c, p] == 0:
                src_arr[c, o:o + q] = ref

    # per-block (over all cores) src min/max for call chunking
    src_blk = src_arr.reshape(N_CORES, slot_total // 128, 128)
    blk_min = src_blk.min(axis=(0, 2))
    blk_max = src_blk.max(axis=(0, 2))
    cnt_min, cnt_max = [], []
    bo = 0
    for p in range(NT):
        nb = int(quota[p]) // 128
        cnt_min.append(blk_min[bo:bo + nb])
        cnt_max.append(blk_max[bo:bo + nb])
        bo += nb
    calls, blk_tile, sched_total = _build_schedule(cnt_min, cnt_max, quota)
    assert sched_total == slot_total

    # idx values per call: src - base
    idx_arr = np.zeros((N_CORES, slot_total), np.int16)
    for o, q, base in calls:
        seg = src_arr[:, o:o + q] - base
        assert seg.min() >= 0 and seg.max() <= SPAN, (seg.min(), seg.max())
        idx_arr[:, o:o + q] = seg.astype(np.int16)

    # idx wrapped [16, Q/16] replicated to 128 partitions; dl wrapped
    # [128, Q/128]
    idxcols = slot_total // 16
    nb_tot = slot_total // 128
    idx_sb = (idx_arr.reshape(N_CORES, nb_tot * 8, 16)
              .transpose(0, 2, 1))
    idx_sb = np.ascontiguousarray(np.tile(idx_sb, (1, 8, 1)))
    dl_sb = np.ascontiguousarray(
        dl_arr.reshape(N_CORES, nb_tot, 128).transpose(0, 2, 1))

    # dense inputs: xs = dinv * x in the NEW node order
    xs = np.zeros((NPAD, D_IN), np.float32)
    xs[node_newpos[:N]] = dinv[:, None] * x
    xs = xs.astype(NP_BF16)

    dinv_new = np.zeros(NPAD, np.float32)
    dinv_new[node_newpos] = dinv_pad
    dinv_dst = np.stack([
        np.ascontiguousarray(
            dinv_new[c * SHARD:(c + 1) * SHARD].reshape(NT, 128).T)
        for c in range(N_CORES)])  # [8, 128, NT]

    iota = np.tile(np.arange(128), (128, 1)).astype(NP_BF16)
    identb = np.eye(128, dtype=np.float32).astype(NP_BF16)
    b1bc = np.ascontiguousarray(
        np.broadcast_to(np.asarray(b1, np.float32), (128, D_HID)))
    b2bc = np.ascontiguousarray(
        np.broadcast_to(np.asarray(b2, np.float32), (128, D_OUT)))

    common = dict(xs=xs, W1=np.asarray(W1, NP_BF16),
                  W2=np.asarray(W2, NP_BF16), b1bc=b1bc, b2bc=b2bc,
                  iota=iota, identb=identb)
    in_maps = []
    for c in range(N_CORES):
        m = dict(common)
        m["dinv_dst"] = np.ascontiguousarray(dinv_dst[c])
        m["idx_sb"] = np.ascontiguousarray(idx_sb[c])
        m["dstloc"] = np.ascontiguousarray(dl_sb[c])
        in_maps.append(m)

    meta = dict(quota=quota, idxcols=idxcols, nb=nb_tot, calls=calls,
                blk_tile=blk_tile, node_newpos=node_newpos)
    return in_maps, meta


# --------------------------------------------------------------------------
# Device program
# --------------------------------------------------------------------------

def build_program(cfg, meta, with_collective=True, phases=(1, 2, 3)):
    NPAD, NT, SHARD = cfg["NPAD"], cfg["NT"], cfg["SHARD"]
    D_IN, D_HID, D_OUT = cfg["D_IN"], cfg["D_HID"], cfg["D_OUT"]
    idxcols, nb = meta["idxcols"], meta["nb"]
    calls, blk_tile = meta["calls"], meta["blk_tile"]
    KI = D_IN // 128   # k-chunks for W1 matmul
    KH = D_HID // 128  # k-chunks for W2 matmul
    CMAX = QMAX // 128
    D_L2 = 2 * D_OUT   # layer-2 table row: 64 bf16 data + 64 bf16 zeros

    first_blk = {}
    last_blk = {}
    for i, t in enumerate(blk_tile):
        first_blk.setdefault(t, i)
        last_blk[t] = i

    nc = bacc.Bacc("TRN2", target_bir_lowering=False, debug=False,
                   num_devices=N_CORES)

    xs_d = nc.dram_tensor("xs", [NPAD, D_IN], BF16, kind="ExternalInput")
    W1_d = nc.dram_tensor("W1", [D_IN, D_HID], BF16, kind="ExternalInput")
    W2_d = nc.dram_tensor("W2", [D_HID, D_OUT], BF16, kind="ExternalInput")
    b1_d = nc.dram_tensor("b1bc", [128, D_HID], F32, kind="ExternalInput")
    b2_d = nc.dram_tensor("b2bc", [128, D_OUT], F32, kind="ExternalInput")
    iota_d = nc.dram_tensor("iota", [128, 128], BF16, kind="ExternalInput")
    identb_d = nc.dram_tensor("identb", [128, 128], BF16,
                              kind="ExternalInput")
    dinvd_d = nc.dram_tensor("dinv_dst", [128, NT], F32, kind="ExternalInput")
    idx_d = nc.dram_tensor("idx_sb", [128, idxcols], I16, kind="ExternalInput")
    dl_d = nc.dram_tensor("dstloc", [128, nb], F32, kind="ExternalInput")
    out_d = nc.dram_tensor("out", [SHARD, D_OUT], F32, kind="ExternalOutput")

    with tile.TileContext(nc) as tc:
        with (
            tc.tile_pool(name="const", bufs=1) as const,
            tc.tile_pool(name="dram", bufs=1, space="DRAM") as dram,
            tc.tile_pool(name="mt", bufs=10) as mpool,
            tc.tile_pool(name="st", bufs=8) as spool,
            tc.tile_pool(name="ag1ps", bufs=4, space="PSUM") as apsum1,
            tc.tile_pool(name="ag2ps", bufs=3, space="PSUM") as apsum2,
            tc.tile_pool(name="ep", bufs=3) as eppool,
            tc.tile_pool(name="ep2", bufs=1, space="PSUM") as eppsum,
        ):
            h2own = dram.tile([SHARD, D_L2], BF16)
            h2full = dram.tile([NPAD, D_L2], BF16, addr_space="Shared")

            w1_sb = const.tile([128, KI, D_HID], BF16)
            for k in range(KI):
                nc.sync.dma_start(out=w1_sb[:, k, :],
                                  in_=W1_d.ap()[k * 128:(k + 1) * 128, :])
            w2_sb = const.tile([128, KH, D_OUT], BF16)
            for k in range(KH):
                nc.sync.dma_start(out=w2_sb[:, k, :],
                                  in_=W2_d.ap()[k * 128:(k + 1) * 128, :])
            iota_sb = const.tile([128, 128], BF16)
            nc.sync.dma_start(out=iota_sb[:], in_=iota_d.ap())
            identb_sb = const.tile([128, 128], BF16)
            nc.sync.dma_start(out=identb_sb[:], in_=identb_d.ap())
            b1_sb = const.tile([128, D_HID], F32)
            nc.sync.dma_start(out=b1_sb[:], in_=b1_d.ap())
            b2_sb = const.tile([128, D_OUT], F32)
            nc.sync.dma_start(out=b2_sb[:], in_=b2_d.ap())
            dinvd_sb = const.tile([128, NT], F32)
            nc.sync.dma_start(out=dinvd_sb[:], in_=dinvd_d.ap())
            idx_sb = const.tile([128, idxcols], I16)
            nc.sync.dma_start(out=idx_sb[:], in_=idx_d.ap())
            dl_sb = const.tile([128, nb], F32)
            nc.sync.dma_start(out=dl_sb[:], in_=dl_d.ap())

            # deferred log_softmax state
            t0_all = const.tile([128, NT, D_OUT], F32)
            nm_all = const.tile([128, NT], F32)
            se_all = const.tile([128, NT], F32)
            ls_all = const.tile([128, NT], F32)
            out_all = const.tile([128, NT, D_OUT], F32)

            h2own_r = h2own.rearrange("(t p) f -> t p f", p=128)
            out_rt = out_d.ap().rearrange("(t p) f -> p t f", p=128)

            def agg_phase(table_row0, elem, rhs_w, epilogue, mtag, apsum):
                """Gather + one-hot-S + matmul accumulation over the call
                schedule.  table_row0(base) -> DRAM AP starting at row
                base."""
                blk = 0
                psums = {}
                for o, q, base in calls:
                    ncols = q // 128
                    mt = mpool.tile([128, CMAX, elem], BF16, tag=mtag)
                    nc.gpsimd.dma_gather(
                        mt[:, :ncols, :],
                        table_row0(base),
                        idx_sb[:, o // 16:(o + q) // 16],
                        q, q, elem)
                    for j in range(ncols):
                        t = blk_tile[blk]
                        if blk == first_blk[t]:
                            psums[t] = apsum.tile(
                                [128, rhs_w], F32, tag="agg" + mtag,
                                name="aggps")
                        st = spool.tile([128, 128], BF16, tag="s",
                                        name="stile")
                        nc.vector.tensor_scalar(
                            st[:], iota_sb[:], dl_sb[:, blk:blk + 1],
                            None, ALU.is_equal)
                        nc.tensor.matmul(
                            psums[t][:], st[:], mt[:, j, :rhs_w],
                            start=(blk == first_blk[t]),
                            stop=(blk == last_blk[t]))
                        if blk == last_blk[t]:
                            epilogue(t, psums.pop(t))
                        blk += 1

            # ---- layer-1 epilogue: h1 = relu(dinvd*(agg@W1)+b1);
            #      h2row = dinvd*(h1@W2) -> bf16 shard table ----
            def epi1(t, ps):
                o0 = eppool.tile([128, D_HID], BF16, tag="o0")
                nc.scalar.activation(o0[:], ps[:], AF.Copy,
                                     scale=dinvd_sb[:, t:t + 1])
                t0sb = eppool.tile([128, KI, 128], BF16, tag="t0sb")
                for k in range(KI):
                    tp = eppsum.tile([128, 128], BF16, tag="tr")
                    nc.tensor.transpose(tp[:], o0[:, k * 128:(k + 1) * 128],
                                        identb_sb[:])
                    nc.vector.tensor_copy(t0sb[:, k, :], tp[:])
                h1p = eppsum.tile([128, D_HID], F32, tag="h1")
                for k in range(KI):
                    nc.tensor.matmul(h1p[:], t0sb[:, k, :], w1_sb[:, k, :],
                                     start=(k == 0), stop=(k == KI - 1))
                nc.vector.tensor_tensor(h1p[:], h1p[:], b1_sb[:], ALU.add)
                o1 = eppool.tile([128, D_HID], BF16, tag="o1")
                nc.scalar.activation(o1[:], h1p[:], AF.Relu)
                t1sb = eppool.tile([128, KH, 128], BF16, tag="t1sb")
                for k in range(KH):
                    tp2 = eppsum.tile([128, 128], BF16, tag="tr")
                    nc.tensor.transpose(tp2[:], o1[:, k * 128:(k + 1) * 128],
                                        identb_sb[:])
                    nc.vector.tensor_copy(t1sb[:, k, :], tp2[:])
                h2p = eppsum.tile([128, D_OUT], F32, tag="h2")
                for k in range(KH):
                    nc.tensor.matmul(h2p[:], t1sb[:, k, :], w2_sb[:, k, :],
                                     start=(k == 0), stop=(k == KH - 1))
                h2sb = eppool.tile([128, D_L2], BF16, tag="h2sb")
                nc.vector.tensor_scalar(h2sb[:, :D_OUT], h2p[:],
                                        dinvd_sb[:, t:t + 1], None, ALU.mult)
                nc.vector.memset(h2sb[:, D_OUT:], 0.0)
                nc.sync.dma_start(out=h2own_r[t], in_=h2sb[:])

            if 2 in phases:
                agg_phase(
                    lambda base: xs_d.ap()[base:min(base + SPAN + 1, NPAD), :],
                    D_IN, D_IN, epi1, "m1", "s1", "ag1")

            # ---- AllGather h2 shards ----
            if with_collective and 2 in phases:
                nc.gpsimd.collective_compute(
                    "AllGather", ALU.bypass,
                    replica_groups=[list(range(N_CORES))],
                    ins=[h2own.opt()], outs=[h2full.opt()])

            # ---- layer-2 aggregation; exp inline, ln/sub deferred ----
            def epi2(t, ps, eppool, eppsum):
                nc.vector.tensor_scalar(t0_all[:, t, :], ps[:],
                                        dinvd_sb[:, t:t + 1], None, ALU.mult)
                nc.vector.tensor_tensor(t0_all[:, t, :], t0_all[:, t, :],
                                        b2_sb[:], ALU.add)
                nc.vector.tensor_reduce(nm_all[:, t:t + 1], t0_all[:, t, :],
                                        mybir.AxisListType.X, ALU.max,
                                        negate=True)
                et = eppool.tile([128, D_OUT], F32, tag="et")
                nc.scalar.activation(et[:], t0_all[:, t, :], AF.Exp,
                                     bias=nm_all[:, t:t + 1],
                                     accum_out=se_all[:, t:t + 1])

            if 3 in phases:
                agg_phase(
                    lambda base: h2full[base:min(base + SPAN + 1, NPAD), :],
                    D_L2, D_OUT, epi2, "m2", "s2", "ag2")

                with tc.tile_pool(name="fin", bufs=4) as finpool:
                    nc.scalar.activation(ls_all[:], se_all[:], AF.Ln)
                    for t in range(NT):
                        ot = finpool.tile([128, D_OUT], F32, tag="ot")
                        nc.vector.tensor_scalar(
                            ot[:], t0_all[:, t, :], nm_all[:, t:t + 1],
                            ls_all[:, t:t + 1], ALU.add, ALU.subtract)
                        nc.sync.dma_start(out=out_r[t], in_=ot[:])

    nc.compile()
    return nc


# --------------------------------------------------------------------------
# Entry point
# --------------------------------------------------------------------------

def kernel(x, edge_index, W1, b1, W2, b2):
    cfg = FULL_CFG
    in_maps, meta = preprocess(x, edge_index, W1, b1, W2, b2, cfg)
    nc = build_program(cfg, meta)
    res = run_bass_kernel_spmd(nc, in_maps, core_ids=list(range(N_CORES)))
    shards = [res.results[c]["out"] for c in range(N_CORES)]
    full = np.concatenate(shards, axis=0)          # rows in NEW node order
    out = full[meta["node_newpos"][: cfg["N"]]]
    return np.ascontiguousarray(out.astype(np.float32))
